# revision 40
# baseline (speedup 1.0000x reference)
"""Trainium2 Bass kernel for ClassicAttention (B=2, S=2048, D=1024, H=16).

Sharding: tensor-parallel over heads across 8 cores (2 heads/core), with
x^T replicated to every core (pre-transposed + pre-cast to bf16 on the
host, so no on-device gather/transpose/cast is on the critical path --
the first all-core collective can't complete before ~90us due to NEFF
start skew, so the front of the pipeline must not depend on one).
  - QKV projection: each core computes Q^T,K^T (d-major) and V (row-major)
    for its 2 heads over all B*S rows straight from the replicated x^T.
  - Attention: transposed-scores formulation S^T[k,q] so the softmax exp
    output is directly P^T (the AV matmul's moving operand); the softmax
    denominator comes from a ones-column appended to V (row 64 of the AV
    accumulator). No max-subtraction (scores bounded ~|3.3| here).
  - Normalize per q-super directly from PSUM: reciprocal (Vector) +
    partition_broadcast (GpSimd) -- no DRAM bounce.
  - c_proj: per-(batch, q-super) ctx AllGather chunks; each chunk's
    c_proj is deferred two attention iterations so the gather overlaps
    compute. Output is transposed ([j, B*S]); the host transposes back.
All matmuls bf16 inputs with fp32 PSUM accumulation.
"""

import numpy as np
import ml_dtypes

import concourse.bass as bass
import concourse.tile as tile
import concourse.mybir as mybir
from concourse import bacc, bass_isa
from concourse.bass_utils import run_bass_kernel_spmd

F32 = mybir.dt.float32
BF16 = mybir.dt.bfloat16

NCORES = 8
B, S, D = 2, 2048, 1024
H, HD = 16, 64
HPC = H // NCORES          # heads per core = 2
M = B * S                  # 4096 rows
NSUP = M // 512            # 8 row-supers of 512
ST_B = S // 128            # 16 s-tiles per batch
KCH = D // 128             # 8 contraction chunks
G_PER_B = S // 512         # 4 q-supers per batch
SCALE = 1.0 / (HD ** 0.5)


def build_ir(nc):
    # ---------------- DRAM I/O ----------------
    xT = nc.dram_tensor("xT", [D, M], BF16, kind="ExternalInput").ap()
    wqk = nc.dram_tensor("wqk", [D, 256], BF16, kind="ExternalInput").ap()
    wv = nc.dram_tensor("wv", [D, 128], BF16, kind="ExternalInput").ap()
    wp = nc.dram_tensor("wp", [D, 128], BF16, kind="ExternalInput").ap()
    bqk = nc.dram_tensor("bqk", [256], F32, kind="ExternalInput").ap()
    bv = nc.dram_tensor("bv", [128], F32, kind="ExternalInput").ap()
    bp = nc.dram_tensor("bp", [128], F32, kind="ExternalInput").ap()
    outT = nc.dram_tensor("outT", [128, M], F32, kind="ExternalOutput").ap()

    # causal mask master: Mm[k, c] = 1 if c >= k + 384 else 0  (bf16)
    mask_np = (np.arange(896)[None, :] >= (np.arange(128)[:, None] + 384))
    mask_const = nc.inline_tensor(mask_np.astype(ml_dtypes.bfloat16), "mask_const").ap()

    rg = [list(range(NCORES))]

    with tile.TileContext(nc) as tc:
        _emit(nc, tc, xT, wqk, wv, wp, bqk, bv, bp, outT, mask_const, rg)
    return nc


def _emit(nc, tc, xT, wqk, wv, wp, bqk, bv, bp, outT, mask_const, rg):
    import contextlib
    es = contextlib.ExitStack()
    with es:
        singles = es.enter_context(tc.tile_pool(name="singles", bufs=1))
        dram = es.enter_context(tc.tile_pool(name="dram", bufs=1, space="DRAM"))

        # Tiny dummy AllGather issued first: absorbs the one-time collective
        # rendezvous / NEFF start skew in parallel with the local prologue.
        dummy_in = dram.tile([1, 16], BF16, tag="dummy_in", name="dummy_in")
        dummy_out = dram.tile([NCORES, 16], BF16, addr_space="Shared",
                              tag="dummy_out", name="dummy_out")
        nc.gpsimd.collective_compute(
            "AllGather", mybir.AluOpType.bypass, replica_groups=rg,
            ins=[dummy_in.opt()], outs=[dummy_out.opt()],
        )

        # ------------- persistent SBUF -------------
        qT = singles.tile([128, M], BF16)          # [2 heads x 64 d, B*S]
        kT = singles.tile([128, M], BF16)
        v_sb = singles.tile([128, B * ST_B, 128], BF16)  # [Va(64)|Vb(64)] per s-tile
        mask_sb = singles.tile([128, 896], BF16)
        nc.sync.dma_start(out=mask_sb, in_=mask_const)

        # weights (already bf16, straight from DRAM)
        wqk_sb = singles.tile([128, KCH, 256], BF16)
        wv_sb = singles.tile([128, KCH, 128], BF16)
        wp_sb = singles.tile([128, KCH, 128], BF16)
        bqk_sb = singles.tile([128, 2], F32)
        bp_sb = singles.tile([128, 1], F32)
        bv_bc = singles.tile([128, 128], F32)
        ones_row = singles.tile([1, 128], F32)
        bv_row = singles.tile([1, 128], F32)
        nc.vector.memset(ones_row, 1.0)
        nc.sync.dma_start(out=wqk_sb, in_=wqk.rearrange("(c p) j -> p c j", p=128))

        # ------------- replicated x^T -> SBUF -------------
        # batch-0 tiles issued before the bulky wv/wp loads so QKV(b0)
        # starts as early as possible
        xt = {}
        xt_pool = es.enter_context(tc.tile_pool(name="xt", bufs=B * KCH))
        xv = xT.rearrange("(c p) m -> c p m", p=128)
        for c in range(KCH):
            xtile = xt_pool.tile([128, S], BF16, tag="xtile")
            nc.sync.dma_start(out=xtile, in_=xv[c][:, 0:S])
            xt[(0, c)] = xtile
        nc.sync.dma_start(out=wv_sb, in_=wv.rearrange("(c p) j -> p c j", p=128))
        nc.sync.dma_start(out=bqk_sb, in_=bqk.rearrange("(t p) -> p t", p=128))
        nc.sync.dma_start(out=bp_sb, in_=bp.rearrange("(a p) -> p a", p=128))
        nc.sync.dma_start(out=bv_row, in_=bv.rearrange("(a j) -> a j", a=1))
        for c in range(KCH):
            xtile = xt_pool.tile([128, S], BF16, tag="xtile")
            nc.sync.dma_start(out=xtile, in_=xv[c][:, S:2 * S])
            xt[(1, c)] = xtile
        nc.sync.dma_start(out=wp_sb, in_=wp.rearrange("(c p) j -> p c j", p=128))

        with tc.tile_pool(name="bias_ps", bufs=1, space="PSUM") as bias_ps:
            # bv broadcast tile: outer(ones[128], bv[128]) via K=1 matmul
            bvp = bias_ps.tile([128, 128], F32)
            nc.tensor.matmul(bvp, lhsT=ones_row, rhs=bv_row, start=True, stop=True)
            nc.vector.tensor_copy(bv_bc, bvp)

        pt_pool = es.enter_context(tc.tile_pool(name="pt", bufs=4))
        post = es.enter_context(tc.tile_pool(name="post", bufs=2))

        # ------------- phase 2: QKV projection -------------
        with tc.tile_pool(name="qk_ps", bufs=2, space="PSUM") as qk_ps, \
             tc.tile_pool(name="v_ps", bufs=2, space="PSUM") as v_ps:

            for su in range(NSUP):
                b = su // (NSUP // B)
                mo = (su % (NSUP // B)) * 512  # column offset within batch
                # Q^T and K^T for this row-super (d-major, both heads stacked)
                for jt, dst in ((0, qT), (1, kT)):
                    ps = qk_ps.tile([128, 512], F32, tag="qk")
                    for kc in range(KCH):
                        nc.tensor.matmul(
                            ps,
                            lhsT=wqk_sb[:, kc, jt * 128:(jt + 1) * 128],
                            rhs=xt[(b, kc)][:, mo:mo + 512],
                            start=(kc == 0), stop=(kc == KCH - 1),
                        )
                    nc.vector.tensor_scalar_add(
                        dst[:, su * 512:(su + 1) * 512], ps, bqk_sb[:, jt:jt + 1])
                # V (row-major) for the 4 s-tiles of this super
                for mt in range(4):
                    st = su * 4 + mt   # global s-tile index (b*16 + in-batch tile)
                    ps = v_ps.tile([128, 128], F32, tag="v")
                    for kc in range(KCH):
                        nc.tensor.matmul(
                            ps,
                            lhsT=xt[(b, kc)][:, mo + mt * 128:mo + (mt + 1) * 128],
                            rhs=wv_sb[:, kc, :],
                            start=(kc == 0), stop=(kc == KCH - 1),
                        )
                    nc.vector.tensor_add(v_sb[:, st, :], ps, bv_bc)

            # (qk/v psum pools close here, freeing banks for attention)

        # ------------- phase 3: attention + chunked c_proj -------------
        cs_pool = es.enter_context(tc.tile_pool(name="cs", bufs=4))
        acc_pool = es.enter_context(tc.tile_pool(name="acc", bufs=2))
        EXP = mybir.ActivationFunctionType.Exp
        pending = []
        with tc.tile_pool(name="s_ps", bufs=2, space="PSUM") as s_ps, \
             tc.tile_pool(name="ctx_ps", bufs=2, space="PSUM") as ctx_ps, \
             tc.tile_pool(name="cp_ps", bufs=1, space="PSUM") as cp_ps, \
             tc.tile_pool(name="cg", bufs=5) as cg_pool, \
             tc.tile_pool(name="osb", bufs=3) as osb:

            def emit_cg_prefetch(b, g, ag_tile):
                # issue the gathered-ctx SBUF loads well before the matmuls
                cgs = []
                for c in range(NCORES):
                    cg = cg_pool.tile([128, 512], BF16, tag="cg",
                                      name=f"cg{c}")
                    nc.sync.dma_start(out=cg, in_=ag_tile[c])
                    cgs.append(cg)
                return cgs

            def emit_cproj(b, g, cgs):
                ps = cp_ps.tile([128, 512], F32, tag="cp")
                for c in range(NCORES):
                    nc.tensor.matmul(
                        ps, lhsT=wp_sb[:, c, :], rhs=cgs[c],
                        start=(c == 0), stop=(c == NCORES - 1),
                    )
                o = osb.tile([128, 512], F32, tag="o")
                nc.vector.tensor_scalar_add(o, ps, bp_sb)
                nc.sync.dma_start(
                    out=outT[:, b * S + g * 512:(b * S + (g + 1) * 512)],
                    in_=o)

            # interleave batches (b0g0, b1g0, b0g1, ...) so ctx AllGathers
            # spread evenly; early blocks emit no c_proj (their gathers are
            # delayed by core start skew), later blocks drain progressively.
            # cg loads prefetch one block ahead of the matmuls (but no
            # earlier than block 4, when the skewed early gathers are done).
            blocks = [(b, g) for g in range(G_PER_B) for b in range(B)]
            lag_sched = [99, 99, 99, 99, 3, 3, 2, 1]
            for bi, (b, g) in enumerate(blocks):
                    for item in pending:
                        if item[3] is None and bi >= max(item[4] + 2, 4):
                            item[3] = emit_cg_prefetch(item[0], item[1], item[2])
                    lag = lag_sched[bi]
                    while len(pending) > lag:
                        item = pending.pop(0)
                        if item[3] is None:
                            item[3] = emit_cg_prefetch(item[0], item[1], item[2])
                        emit_cproj(item[0], item[1], item[3])
                    n_kt = 4 * g + 4
                    # single [128, 512] PSUM tile: head0 ctx in partitions
                    # 0-63, head1 in 64-127 (column-tiled AV pair)
                    cps = ctx_ps.tile([128, 512], F32, tag="ctx")
                    pta = [acc_pool.tile([128, 512], F32, tag=f"pta{_hl}",
                                         name=f"pta{_hl}")
                           for _hl in range(HPC)]
                    q_sl = [qT[hl * 64:(hl + 1) * 64,
                               b * S + g * 512:b * S + (g + 1) * 512]
                            for hl in range(HPC)]
                    n_kp = n_kt // 2

                    def emit_scores(kp):
                        # scores: alternate heads so the two K=64 matmuls
                        # share the PE array (row groups 0-1 / 2-3)
                        sps = [s_ps.tile([128, 1024], F32, tag="s",
                                         name=f"sps{_hl}")
                               for _hl in range(HPC)]
                        pts = [pt_pool.tile([128, 1024], BF16, tag="pt",
                                            name=f"pt{_hl}")
                               for _hl in range(HPC)]
                        for half in (0, 1):
                            kt = 2 * kp + half
                            qo = max(kt - 4 * g, 0) * 128  # causal trim
                            for hl in range(HPC):
                                nc.tensor.matmul(
                                    sps[hl][:, half * 512 + qo:(half + 1) * 512],
                                    lhsT=kT[hl * 64:(hl + 1) * 64,
                                            b * S + kt * 128:b * S + (kt + 1) * 128],
                                    rhs=q_sl[hl][:, qo:512],
                                    start=True, stop=True,
                                    tile_position=(64 * hl, 0),
                                )
                        return sps, pts

                    def emit_exp(kp, sps, pts):
                        for hl in range(HPC):
                            pt, sp = pts[hl], sps[hl]
                            if 2 * kp + 1 < 4 * g:        # both halves full
                                nc.scalar.activation(pt, sp, EXP, scale=SCALE)
                            else:                          # diagonal pair
                                for half in (0, 1):
                                    kt = 2 * kp + half
                                    qo = max(kt - 4 * g, 0) * 128
                                    lo = half * 512 + qo
                                    if qo > 0:
                                        nc.vector.memset(
                                            pt[:, half * 512:lo], 0.0)
                                    nc.scalar.activation(
                                        pt[:, lo:(half + 1) * 512],
                                        sp[:, lo:(half + 1) * 512],
                                        EXP, scale=SCALE)
                                    if kt - 4 * g >= 0:
                                        nc.vector.tensor_mul(
                                            pt[:, lo:lo + 128],
                                            pt[:, lo:lo + 128],
                                            mask_sb[:, 384:512])

                    def emit_av(kp, pts):
                        # both heads' AV column-tiled into one array pass
                        for half in (0, 1):
                            kt = 2 * kp + half
                            for hl in range(HPC):
                                nc.tensor.matmul(
                                    cps[hl * 64:(hl + 1) * 64, :],
                                    lhsT=v_sb[:, b * ST_B + kt,
                                              hl * 64:(hl + 1) * 64],
                                    rhs=pts[hl][:, half * 512:(half + 1) * 512],
                                    start=(kt == 0), stop=(kt == n_kt - 1),
                                    tile_position=(0, 64 * hl),
                                )

                    def emit_acc(kp, pts):
                        # softmax denominators: accumulate exp into f32 on DVE
                        for hl in range(HPC):
                            if kp == 0:
                                nc.vector.tensor_add(
                                    pta[hl], pts[hl][:, 0:512],
                                    pts[hl][:, 512:1024])
                            else:
                                for half in (0, 1):
                                    nc.vector.tensor_add(
                                        pta[hl], pta[hl],
                                        pts[hl][:, half * 512:(half + 1) * 512])

                    # software pipeline: scores(kp+1) issued before AV(kp)
                    # so the Tensor queue never head-of-line blocks on exp
                    tiles = {0: emit_scores(0)}
                    for kp in range(n_kp):
                        if kp + 1 < n_kp:
                            tiles[kp + 1] = emit_scores(kp + 1)
                        sps, pts = tiles.pop(kp)
                        emit_exp(kp, sps, pts)
                        emit_av(kp, pts)
                        emit_acc(kp, pts)
                    # ---- normalize straight from PSUM, per q-super ----
                    ctx_loc = dram.tile([128, 512], BF16, tag="ctx_loc", bufs=2,
                                        name=f"ctx_loc{b}_{g}")
                    ctx_ag = dram.tile([NCORES, 128, 512], BF16,
                                       addr_space="Shared", tag="ctx_ag",
                                       name=f"ctx_ag{b}_{g}")
                    for hl in range(HPC):
                        red = post.tile([128, 512], F32, tag=f"red{hl}")
                        nc.gpsimd.partition_all_reduce(
                            red, pta[hl], channels=128,
                            reduce_op=bass_isa.ReduceOp.add)
                        rc = post.tile([1, 512], F32, tag=f"rc{hl}")
                        nc.vector.reciprocal_approx_fast(rc, red[0:1, :])
                        bc = post.tile([64, 512], F32, tag=f"bc{hl}")
                        nc.gpsimd.partition_broadcast(bc, rc, channels=64)
                        cs = cs_pool.tile([64, 512], BF16, tag=f"cs{hl}")
                        nc.vector.tensor_mul(
                            cs, cps[hl * 64:(hl + 1) * 64, :], bc)
                        nc.sync.dma_start(
                            out=ctx_loc[hl * 64:(hl + 1) * 64, :], in_=cs)
                    nc.gpsimd.collective_compute(
                        "AllGather", mybir.AluOpType.bypass, replica_groups=rg,
                        ins=[ctx_loc.opt()], outs=[ctx_ag.opt()],
                    )
                    pending.append([b, g, ctx_ag, None, bi])
            for item in pending:
                if item[3] is None:
                    item[3] = emit_cg_prefetch(item[0], item[1], item[2])
                emit_cproj(item[0], item[1], item[3])


_CACHE = {}


def _get_compiled():
    if "nc" not in _CACHE:
        nc = bacc.Bacc("TRN2", target_bir_lowering=False, debug=False,
                       num_devices=NCORES)
        build_ir(nc)
        nc.compile()
        _CACHE["nc"] = nc
    return _CACHE["nc"]


def make_in_maps(inputs):
    x = np.asarray(inputs["hidden_states"], dtype=np.float32)   # [B,S,D]
    wa = np.asarray(inputs["c_attn_w"], dtype=np.float32)       # [D, 3D]
    ba = np.asarray(inputs["c_attn_b"], dtype=np.float32)       # [3D]
    wpr = np.asarray(inputs["c_proj_w"], dtype=np.float32)      # [D, D]
    bpr = np.asarray(inputs["c_proj_b"], dtype=np.float32)      # [D]

    xf = np.ascontiguousarray(x.reshape(M, D))
    xTh = np.ascontiguousarray(xf.T).astype(ml_dtypes.bfloat16)  # [D, M]
    wq, wk, wv_full = wa[:, 0:D], wa[:, D:2 * D], wa[:, 2 * D:3 * D]
    bq, bk, bv_full = ba[0:D], ba[D:2 * D], ba[2 * D:3 * D]

    in_maps = []
    for r in range(NCORES):
        hs = slice(r * HPC * HD, (r + 1) * HPC * HD)   # this core's head dims
        in_maps.append({
            "xT": xTh,
            "wqk": np.ascontiguousarray(
                np.concatenate([wq[:, hs], wk[:, hs]],
                               axis=1)).astype(ml_dtypes.bfloat16),
            "wv": np.ascontiguousarray(wv_full[:, hs]).astype(ml_dtypes.bfloat16),
            "wp": np.ascontiguousarray(
                wpr[:, r * 128:(r + 1) * 128]).astype(ml_dtypes.bfloat16),
            "bqk": np.ascontiguousarray(np.concatenate([bq[hs], bk[hs]])),
            "bv": np.ascontiguousarray(bv_full[hs]),
            "bp": np.ascontiguousarray(bpr[r * 128:(r + 1) * 128]),
        })
    return in_maps


def assemble(results):
    slices = [results[r]["outT"].T.reshape(B, S, 128) for r in range(NCORES)]
    return np.ascontiguousarray(np.concatenate(slices, axis=2).astype(np.float32))


def kernel(**inputs):
    in_maps = make_in_maps(inputs)
    nc = _get_compiled()
    res = run_bass_kernel_spmd(nc, in_maps, core_ids=list(range(NCORES)))
    return assemble(res.results)


if __name__ == "__main__":
    import reference
    inp = reference.setup_inputs()
    out = kernel(**{k: np.asarray(v) for k, v in inp.items()})
    print(out.shape, out.dtype)


# revision 42
# speedup vs baseline: 1.0974x; 1.0974x over previous
"""Trainium2 Bass kernel for ClassicAttention (B=2, S=2048, D=1024, H=16).

Sharding: tensor-parallel over heads across 8 cores (2 heads/core), with
x^T replicated to every core (pre-transposed + pre-cast to bf16 on the
host, so no on-device gather/transpose/cast is on the critical path --
the first all-core collective can't complete before ~90us due to NEFF
start skew, so the front of the pipeline must not depend on one).
  - QKV projection: each core computes Q^T,K^T (d-major) and V (row-major)
    for its 2 heads over all B*S rows straight from the replicated x^T.
  - Attention: transposed-scores formulation S^T[k,q] so the softmax exp
    output is directly P^T (the AV matmul's moving operand); the softmax
    denominator comes from a ones-column appended to V (row 64 of the AV
    accumulator). No max-subtraction (scores bounded ~|3.3| here).
  - Normalize per q-super directly from PSUM: reciprocal (Vector) +
    partition_broadcast (GpSimd) -- no DRAM bounce.
  - c_proj: per-(batch, q-super) ctx AllGather chunks; each chunk's
    c_proj is deferred two attention iterations so the gather overlaps
    compute. Output is transposed ([j, B*S]); the host transposes back.
All matmuls bf16 inputs with fp32 PSUM accumulation.
"""

import numpy as np
import ml_dtypes

import concourse.bass as bass
import concourse.tile as tile
import concourse.mybir as mybir
from concourse import bacc
from concourse.bass_utils import run_bass_kernel_spmd

F32 = mybir.dt.float32
BF16 = mybir.dt.bfloat16

NCORES = 8
B, S, D = 2, 2048, 1024
H, HD = 16, 64
HPC = H // NCORES          # heads per core = 2
M = B * S                  # 4096 rows
NSUP = M // 512            # 8 row-supers of 512
ST_B = S // 128            # 16 s-tiles per batch
KCH = D // 128             # 8 contraction chunks
G_PER_B = S // 512         # 4 q-supers per batch
SCALE = 1.0 / (HD ** 0.5)


def build_ir(nc):
    # ---------------- DRAM I/O ----------------
    xT = nc.dram_tensor("xT", [D, M], BF16, kind="ExternalInput").ap()
    wqk = nc.dram_tensor("wqk", [D, 256], BF16, kind="ExternalInput").ap()
    wv = nc.dram_tensor("wv", [D, 128], BF16, kind="ExternalInput").ap()
    wp = nc.dram_tensor("wp", [D, 128], BF16, kind="ExternalInput").ap()
    bqk = nc.dram_tensor("bqk", [256], F32, kind="ExternalInput").ap()
    bv = nc.dram_tensor("bv", [128], F32, kind="ExternalInput").ap()
    bp = nc.dram_tensor("bp", [128], F32, kind="ExternalInput").ap()
    outT = nc.dram_tensor("outT", [128, M], F32, kind="ExternalOutput").ap()

    # causal mask master: Mm[k, c] = 1 if c >= k + 384 else 0  (bf16)
    mask_np = (np.arange(896)[None, :] >= (np.arange(128)[:, None] + 384))
    mask_const = nc.inline_tensor(mask_np.astype(ml_dtypes.bfloat16), "mask_const").ap()

    rg = [list(range(NCORES))]

    with tile.TileContext(nc) as tc:
        _emit(nc, tc, xT, wqk, wv, wp, bqk, bv, bp, outT, mask_const, rg)
    return nc


def _emit(nc, tc, xT, wqk, wv, wp, bqk, bv, bp, outT, mask_const, rg):
    import contextlib
    es = contextlib.ExitStack()
    with es:
        singles = es.enter_context(tc.tile_pool(name="singles", bufs=1))
        dram = es.enter_context(tc.tile_pool(name="dram", bufs=1, space="DRAM"))

        # Tiny dummy AllGather issued first: absorbs the one-time collective
        # rendezvous / NEFF start skew in parallel with the local prologue.
        dummy_in = dram.tile([1, 16], BF16, tag="dummy_in", name="dummy_in")
        dummy_out = dram.tile([NCORES, 16], BF16, addr_space="Shared",
                              tag="dummy_out", name="dummy_out")
        nc.gpsimd.collective_compute(
            "AllGather", mybir.AluOpType.bypass, replica_groups=rg,
            ins=[dummy_in.opt()], outs=[dummy_out.opt()],
        )

        # ------------- persistent SBUF -------------
        qT = singles.tile([128, M], BF16)          # [2 heads x 64 d, B*S]
        kT = singles.tile([128, M], BF16)
        v_sb = singles.tile([128, B * ST_B, 130], BF16)  # [Va(64)|1|Vb(64)|1] per s-tile
        mask_sb = singles.tile([128, 896], BF16)
        nc.sync.dma_start(out=mask_sb, in_=mask_const)
        nc.vector.memset(v_sb, 1.0)                # ones columns pre-set

        # weights (already bf16, straight from DRAM)
        wqk_sb = singles.tile([128, KCH, 256], BF16)
        wv_sb = singles.tile([128, KCH, 128], BF16)
        wp_sb = singles.tile([128, KCH, 128], BF16)
        bqk_sb = singles.tile([128, 2], F32)
        bp_sb = singles.tile([128, 1], F32)
        bv_bc = singles.tile([128, 128], F32)
        ones_row = singles.tile([1, 128], F32)
        bv_row = singles.tile([1, 128], F32)
        nc.vector.memset(ones_row, 1.0)
        nc.sync.dma_start(out=wqk_sb, in_=wqk.rearrange("(c p) j -> p c j", p=128))

        # ------------- replicated x^T -> SBUF -------------
        # batch-0 tiles issued before the bulky wv/wp loads so QKV(b0)
        # starts as early as possible
        xt = {}
        xt_pool = es.enter_context(tc.tile_pool(name="xt", bufs=B * KCH))
        xv = xT.rearrange("(c p) m -> c p m", p=128)
        for c in range(KCH):
            xtile = xt_pool.tile([128, S], BF16, tag="xtile")
            nc.sync.dma_start(out=xtile, in_=xv[c][:, 0:S])
            xt[(0, c)] = xtile
        nc.sync.dma_start(out=wv_sb, in_=wv.rearrange("(c p) j -> p c j", p=128))
        nc.sync.dma_start(out=bqk_sb, in_=bqk.rearrange("(t p) -> p t", p=128))
        nc.sync.dma_start(out=bp_sb, in_=bp.rearrange("(a p) -> p a", p=128))
        nc.sync.dma_start(out=bv_row, in_=bv.rearrange("(a j) -> a j", a=1))
        for c in range(KCH):
            xtile = xt_pool.tile([128, S], BF16, tag="xtile")
            nc.sync.dma_start(out=xtile, in_=xv[c][:, S:2 * S])
            xt[(1, c)] = xtile
        nc.sync.dma_start(out=wp_sb, in_=wp.rearrange("(c p) j -> p c j", p=128))

        with tc.tile_pool(name="bias_ps", bufs=1, space="PSUM") as bias_ps:
            # bv broadcast tile: outer(ones[128], bv[128]) via K=1 matmul
            bvp = bias_ps.tile([128, 128], F32)
            nc.tensor.matmul(bvp, lhsT=ones_row, rhs=bv_row, start=True, stop=True)
            nc.vector.tensor_copy(bv_bc, bvp)

        pt_pool = es.enter_context(tc.tile_pool(name="pt", bufs=6))
        post = es.enter_context(tc.tile_pool(name="post", bufs=2))

        # ------------- phase 2: QKV projection -------------
        with tc.tile_pool(name="qk_ps", bufs=2, space="PSUM") as qk_ps, \
             tc.tile_pool(name="v_ps", bufs=2, space="PSUM") as v_ps:

            for su in range(NSUP):
                b = su // (NSUP // B)
                mo = (su % (NSUP // B)) * 512  # column offset within batch
                # Q^T and K^T for this row-super (d-major, both heads stacked)
                for jt, dst in ((0, qT), (1, kT)):
                    ps = qk_ps.tile([128, 512], F32, tag="qk")
                    for kc in range(KCH):
                        nc.tensor.matmul(
                            ps,
                            lhsT=wqk_sb[:, kc, jt * 128:(jt + 1) * 128],
                            rhs=xt[(b, kc)][:, mo:mo + 512],
                            start=(kc == 0), stop=(kc == KCH - 1),
                        )
                    nc.vector.tensor_scalar_add(
                        dst[:, su * 512:(su + 1) * 512], ps, bqk_sb[:, jt:jt + 1])
                # V (row-major) for the 4 s-tiles of this super
                for mt in range(4):
                    st = su * 4 + mt   # global s-tile index (b*16 + in-batch tile)
                    ps = v_ps.tile([128, 128], F32, tag="v")
                    for kc in range(KCH):
                        nc.tensor.matmul(
                            ps,
                            lhsT=xt[(b, kc)][:, mo + mt * 128:mo + (mt + 1) * 128],
                            rhs=wv_sb[:, kc, :],
                            start=(kc == 0), stop=(kc == KCH - 1),
                        )
                    for hl in range(HPC):
                        nc.vector.tensor_add(
                            v_sb[:, st, hl * 65:hl * 65 + 64],
                            ps[:, hl * 64:(hl + 1) * 64],
                            bv_bc[:, hl * 64:(hl + 1) * 64],
                        )

            # (qk/v psum pools close here, freeing banks for attention)

        # ------------- phase 3: attention + chunked c_proj -------------
        cs_pool = es.enter_context(tc.tile_pool(name="cs", bufs=4))
        EXP = mybir.ActivationFunctionType.Exp
        pending = []
        with tc.tile_pool(name="s_ps", bufs=2, space="PSUM") as s_ps, \
             tc.tile_pool(name="ctx_ps", bufs=2, space="PSUM") as ctx_ps, \
             tc.tile_pool(name="cp_ps", bufs=1, space="PSUM") as cp_ps, \
             tc.tile_pool(name="cg", bufs=5) as cg_pool, \
             tc.tile_pool(name="osb", bufs=3) as osb:

            def emit_cg_prefetch(b, g, ag_tile):
                # issue the gathered-ctx SBUF loads well before the matmuls
                cgs = []
                for c in range(NCORES):
                    cg = cg_pool.tile([128, 512], BF16, tag="cg",
                                      name=f"cg{c}")
                    nc.sync.dma_start(out=cg, in_=ag_tile[c])
                    cgs.append(cg)
                return cgs

            def emit_cproj(b, g, cgs):
                ps = cp_ps.tile([128, 512], F32, tag="cp")
                for c in range(NCORES):
                    nc.tensor.matmul(
                        ps, lhsT=wp_sb[:, c, :], rhs=cgs[c],
                        start=(c == 0), stop=(c == NCORES - 1),
                    )
                o = osb.tile([128, 512], F32, tag="o")
                nc.vector.tensor_scalar_add(o, ps, bp_sb)
                nc.sync.dma_start(
                    out=outT[:, b * S + g * 512:(b * S + (g + 1) * 512)],
                    in_=o)

            # interleave batches (b0g0, b1g0, b0g1, ...) so ctx AllGathers
            # spread evenly; early blocks emit no c_proj (their gathers are
            # delayed by core start skew), later blocks drain progressively.
            # cg loads prefetch one block ahead of the matmuls (but no
            # earlier than block 4, when the skewed early gathers are done).
            blocks = [(b, g) for g in range(G_PER_B) for b in range(B)]
            lag_sched = [99, 99, 99, 99, 3, 3, 2, 1]
            for bi, (b, g) in enumerate(blocks):
                    for item in pending:
                        if item[3] is None and bi >= max(item[4] + 2, 4):
                            item[3] = emit_cg_prefetch(item[0], item[1], item[2])
                    lag = lag_sched[bi]
                    while len(pending) > lag:
                        item = pending.pop(0)
                        if item[3] is None:
                            item[3] = emit_cg_prefetch(item[0], item[1], item[2])
                        emit_cproj(item[0], item[1], item[3])
                    n_kt = 4 * g + 4
                    cps = [ctx_ps.tile([65, 512], F32, tag="ctx", name=f"cps{_hl}")
                           for _hl in range(HPC)]
                    q_sl = [qT[hl * 64:(hl + 1) * 64,
                               b * S + g * 512:b * S + (g + 1) * 512]
                            for hl in range(HPC)]
                    n_kp = n_kt // 2

                    def emit_scores(kp):
                        # scores: alternate heads so the two K=64 matmuls
                        # share the PE array (row groups 0-1 / 2-3)
                        sps = [s_ps.tile([128, 1024], F32, tag="s",
                                         name=f"sps{_hl}")
                               for _hl in range(HPC)]
                        pts = [pt_pool.tile([128, 1024], BF16, tag="pt",
                                            name=f"pt{_hl}")
                               for _hl in range(HPC)]
                        for half in (0, 1):
                            kt = 2 * kp + half
                            qo = max(kt - 4 * g, 0) * 128  # causal trim
                            for hl in range(HPC):
                                nc.tensor.matmul(
                                    sps[hl][:, half * 512 + qo:(half + 1) * 512],
                                    lhsT=kT[hl * 64:(hl + 1) * 64,
                                            b * S + kt * 128:b * S + (kt + 1) * 128],
                                    rhs=q_sl[hl][:, qo:512],
                                    start=True, stop=True,
                                    tile_position=(64 * hl, 0),
                                )
                        return sps, pts

                    def emit_exp(kp, sps, pts):
                        for hl in range(HPC):
                            pt, sp = pts[hl], sps[hl]
                            if 2 * kp + 1 < 4 * g:        # both halves full
                                nc.scalar.activation(pt, sp, EXP, scale=SCALE)
                            else:                          # diagonal pair
                                for half in (0, 1):
                                    kt = 2 * kp + half
                                    qo = max(kt - 4 * g, 0) * 128
                                    lo = half * 512 + qo
                                    if qo > 0:
                                        nc.vector.memset(
                                            pt[:, half * 512:lo], 0.0)
                                    nc.scalar.activation(
                                        pt[:, lo:(half + 1) * 512],
                                        sp[:, lo:(half + 1) * 512],
                                        EXP, scale=SCALE)
                                    if kt - 4 * g >= 0:
                                        nc.vector.tensor_mul(
                                            pt[:, lo:lo + 128],
                                            pt[:, lo:lo + 128],
                                            mask_sb[:, 384:512])

                    def emit_av(kp, pts):
                        for half in (0, 1):
                            kt = 2 * kp + half
                            for hl in range(HPC):
                                nc.tensor.matmul(
                                    cps[hl],
                                    lhsT=v_sb[:, b * ST_B + kt,
                                              hl * 65:hl * 65 + 65],
                                    rhs=pts[hl][:, half * 512:(half + 1) * 512],
                                    start=(kt == 0), stop=(kt == n_kt - 1),
                                )

                    # software pipeline: scores(kp+1) issued before AV(kp)
                    # so the Tensor queue never head-of-line blocks on exp
                    tiles = {0: emit_scores(0)}
                    for kp in range(n_kp):
                        if kp + 1 < n_kp:
                            tiles[kp + 1] = emit_scores(kp + 1)
                        sps, pts = tiles.pop(kp)
                        emit_exp(kp, sps, pts)
                        emit_av(kp, pts)
                    # ---- normalize straight from PSUM, per q-super ----
                    ctx_loc = dram.tile([128, 512], BF16, tag="ctx_loc", bufs=2,
                                        name=f"ctx_loc{b}_{g}")
                    ctx_ag = dram.tile([NCORES, 128, 512], BF16,
                                       addr_space="Shared", tag="ctx_ag",
                                       name=f"ctx_ag{b}_{g}")
                    for hl in range(HPC):
                        sr = post.tile([1, 512], F32, tag=f"sr{hl}")
                        nc.vector.tensor_copy(sr, cps[hl][64:65, :])
                        rc = post.tile([1, 512], F32, tag=f"rc{hl}")
                        nc.vector.reciprocal_approx_fast(rc, sr)
                        bc = post.tile([64, 512], F32, tag=f"bc{hl}")
                        nc.gpsimd.partition_broadcast(bc, rc, channels=64)
                        cs = cs_pool.tile([64, 512], BF16, tag=f"cs{hl}")
                        nc.vector.tensor_mul(cs, cps[hl][0:64, :], bc)
                        nc.sync.dma_start(
                            out=ctx_loc[hl * 64:(hl + 1) * 64, :], in_=cs)
                    nc.gpsimd.collective_compute(
                        "AllGather", mybir.AluOpType.bypass, replica_groups=rg,
                        ins=[ctx_loc.opt()], outs=[ctx_ag.opt()],
                    )
                    pending.append([b, g, ctx_ag, None, bi])
            for item in pending:
                if item[3] is None:
                    item[3] = emit_cg_prefetch(item[0], item[1], item[2])
                emit_cproj(item[0], item[1], item[3])


_CACHE = {}


def _get_compiled():
    if "nc" not in _CACHE:
        nc = bacc.Bacc("TRN2", target_bir_lowering=False, debug=False,
                       num_devices=NCORES)
        build_ir(nc)
        nc.compile()
        _CACHE["nc"] = nc
    return _CACHE["nc"]


def make_in_maps(inputs):
    x = np.asarray(inputs["hidden_states"], dtype=np.float32)   # [B,S,D]
    wa = np.asarray(inputs["c_attn_w"], dtype=np.float32)       # [D, 3D]
    ba = np.asarray(inputs["c_attn_b"], dtype=np.float32)       # [3D]
    wpr = np.asarray(inputs["c_proj_w"], dtype=np.float32)      # [D, D]
    bpr = np.asarray(inputs["c_proj_b"], dtype=np.float32)      # [D]

    xf = np.ascontiguousarray(x.reshape(M, D))
    xTh = np.ascontiguousarray(xf.T).astype(ml_dtypes.bfloat16)  # [D, M]
    wq, wk, wv_full = wa[:, 0:D], wa[:, D:2 * D], wa[:, 2 * D:3 * D]
    bq, bk, bv_full = ba[0:D], ba[D:2 * D], ba[2 * D:3 * D]

    in_maps = []
    for r in range(NCORES):
        hs = slice(r * HPC * HD, (r + 1) * HPC * HD)   # this core's head dims
        in_maps.append({
            "xT": xTh,
            "wqk": np.ascontiguousarray(
                np.concatenate([wq[:, hs], wk[:, hs]],
                               axis=1)).astype(ml_dtypes.bfloat16),
            "wv": np.ascontiguousarray(wv_full[:, hs]).astype(ml_dtypes.bfloat16),
            "wp": np.ascontiguousarray(
                wpr[:, r * 128:(r + 1) * 128]).astype(ml_dtypes.bfloat16),
            "bqk": np.ascontiguousarray(np.concatenate([bq[hs], bk[hs]])),
            "bv": np.ascontiguousarray(bv_full[hs]),
            "bp": np.ascontiguousarray(bpr[r * 128:(r + 1) * 128]),
        })
    return in_maps


def assemble(results):
    slices = [results[r]["outT"].T.reshape(B, S, 128) for r in range(NCORES)]
    return np.ascontiguousarray(np.concatenate(slices, axis=2).astype(np.float32))


def kernel(**inputs):
    in_maps = make_in_maps(inputs)
    nc = _get_compiled()
    res = run_bass_kernel_spmd(nc, in_maps, core_ids=list(range(NCORES)))
    return assemble(res.results)


if __name__ == "__main__":
    import reference
    inp = reference.setup_inputs()
    out = kernel(**{k: np.asarray(v) for k, v in inp.items()})
    print(out.shape, out.dtype)
